# revision 12
# baseline (speedup 1.0000x reference)
"""Weighted-BCE + masked-MSE loss on 8 Trainium2 cores (pure data parallel).

Math (t in {0,1} exactly): let q = |1 - p - t|  (= p when t=1, 1-p when t=0).
  ln(q^2) = 2*ln(q), so one Square+Ln chain replaces both BCE logs:
    class_sum = -[w1*S_tlq + w0*(S_lq - S_tlq)] / 2,
      S_lq  = sum ln(q^2)   (free via ACT accum),
      S_tlq = sum t*ln(q^2) (one DVE dot)
  masked sq: sum (1-t)*dd^2 = S_sq - S_tsq,  dd = ro - rt
  cnt_zeros = N - S_t  (PE matmul ones.T @ t)
Each core reduces its shard to 5 scalars; host combines and applies weights.

Engine mix per tile (DMA is the bottleneck; everything else has slack):
  DMA : p,rt on SP HWDGE; ro on ACT HWDGE; t on Pool SWDGE with f32->bf16 cast
  Pool: s = p + t; dd-subtract on odd tiles (load-balanced with DVE)
  ACT : u2 = Square(1-s); lq2 = Ln(u2) [+accum S_lq]; sq = Square(dd) [+accum S_sq]
  DVE : dd-subtract on even tiles; two bf16 product+accum dots (t.lq2, t.sq)
  PE  : count = ones.T @ t accumulated in PSUM; final partition reduce
"""

import os
import sys

for _p in ("/opt/trn_rl_repo", "/root/.axon_site/_ro/trn_rl_repo"):
    if os.path.isdir(_p) and _p not in sys.path:
        sys.path.insert(0, _p)

import numpy as np

import concourse.bacc as bacc
import concourse.mybir as mybir
from concourse import tile
from concourse.bass_utils import run_bass_kernel_spmd

N = 16777216
NCORES = 8
NSHARD = N // NCORES  # 2097152
P = 128
F = 1024
NT = NSHARD // (P * F)  # 16

_F32 = mybir.dt.float32
_BF16 = mybir.dt.bfloat16

LAST_RESULTS = None  # test harness peeks at exec_time_ns / trace path


def _build_nc():
    AF = mybir.ActivationFunctionType
    OP = mybir.AluOpType
    AX = mybir.AxisListType

    nc = bacc.Bacc(
        "TRN2", target_bir_lowering=False, debug=False, num_devices=NCORES
    )
    p_d = nc.dram_tensor("p", [NT, P, F], _F32, kind="ExternalInput")
    t_d = nc.dram_tensor("t", [NT, P, F], _F32, kind="ExternalInput")
    ro_d = nc.dram_tensor("ro", [NT, P, F], _F32, kind="ExternalInput")
    rt_d = nc.dram_tensor("rt", [NT, P, F], _F32, kind="ExternalInput")
    out_d = nc.dram_tensor("out", [1, 5], _F32, kind="ExternalOutput")

    with tile.TileContext(nc) as tc:
        with (
            tc.tile_pool(name="io", bufs=8) as io,
            tc.tile_pool(name="work", bufs=3) as work,
            tc.tile_pool(name="stats", bufs=1) as stats,
            tc.tile_pool(name="psum", bufs=1, space="PSUM") as psum,
        ):
            acc_tlq = stats.tile([P, NT], _F32)  # sum t*ln(q^2) per tile col
            acc_lq = stats.tile([P, NT], _F32)  # sum ln(q^2)
            acc_sq = stats.tile([P, NT], _F32)  # sum (ro-rt)^2
            acc_tsq = stats.tile([P, NT], _F32)  # sum t*(ro-rt)^2

            ones_f = stats.tile([P, 1], _F32)
            nc.vector.memset(ones_f[:], 1.0)
            ones_bf = stats.tile([P, 1], _BF16)
            nc.vector.memset(ones_bf[:], 1.0)
            junk = stats.tile([P, F], _BF16)  # dead dot output, reused

            psum_cnt = psum.tile([1, 512], _F32)
            NCHUNK = F // 512

            for i in range(NT):
                tp = io.tile([P, F], _F32, tag="p")
                tt = io.tile([P, F], _BF16, tag="t")
                tro = io.tile([P, F], _F32, tag="ro")
                trt = io.tile([P, F], _F32, tag="rt")
                nc.sync.dma_start(tp[:], p_d[i, :, :])
                nc.gpsimd.dma_start(tt[:], t_d[i, :, :])  # SWDGE f32->bf16 cast
                nc.scalar.dma_start(tro[:], ro_d[i, :, :])
                nc.sync.dma_start(trt[:], rt_d[i, :, :])

                # Pool: s = p + t
                s = work.tile([P, F], _F32, tag="s")
                nc.gpsimd.tensor_add(s[:], tp[:], tt[:])

                # ACT: u2 = (1-s)^2 = q^2 ; lq2 = ln(q^2) with free accum
                u2 = work.tile([P, F], _BF16, tag="u2")
                nc.scalar.activation(u2[:], s[:], AF.Square, bias=1.0, scale=-1.0)
                lq2 = work.tile([P, F], _BF16, tag="lq2")
                nc.scalar.activation(
                    lq2[:], u2[:], AF.Ln, accum_out=acc_lq[:, i : i + 1]
                )

                # dd = ro - rt, alternating engine to balance load
                dd = work.tile([P, F], _BF16, tag="dd")
                nc.vector.tensor_sub(dd[:], tro[:], trt[:])

                # DVE: dot t.lq2
                nc.vector.scalar_tensor_tensor(
                    junk[:], tt[:], 1.0, lq2[:],
                    OP.mult, OP.mult, accum_out=acc_tlq[:, i : i + 1],
                )

                # ACT: sq = dd^2 with free accum; DVE: dot t.sq
                sq = work.tile([P, F], _BF16, tag="sq")
                nc.scalar.activation(
                    sq[:], dd[:], AF.Square, accum_out=acc_sq[:, i : i + 1]
                )
                nc.vector.scalar_tensor_tensor(
                    junk[:], tt[:], 1.0, sq[:],
                    OP.mult, OP.mult, accum_out=acc_tsq[:, i : i + 1],
                )

                # PE: accumulate column-sums of t into psum_cnt
                for c in range(NCHUNK):
                    nc.tensor.matmul(
                        psum_cnt[0:1, :],
                        ones_bf[:, 0:1],
                        tt[:, c * 512 : (c + 1) * 512],
                        start=(i == 0 and c == 0),
                        stop=(i == NT - 1 and c == NCHUNK - 1),
                    )

            # Fold per-tile partials into out[1,5]
            red = stats.tile([P, 4], _F32)
            for j, acc in enumerate((acc_tlq, acc_lq, acc_sq, acc_tsq)):
                nc.vector.tensor_reduce(red[:, j : j + 1], acc[:], AX.X, OP.add)
            psum_fin = psum.tile([1, 4], _F32)
            nc.tensor.matmul(
                psum_fin[0:1, :], ones_f[:, 0:1], red[:, 0:4],
                start=True, stop=True,
            )
            out_sb = stats.tile([P, 8], _F32)
            nc.vector.tensor_scalar_add(out_sb[0:1, 0:4], psum_fin[0:1, :], 0.0)
            nc.vector.tensor_reduce(out_sb[0:1, 4:5], psum_cnt[0:1, :], AX.X, OP.add)
            nc.sync.dma_start(out_d[:], out_sb[0:1, 0:5])

    # Bacc pipeline: splits multi-wait sync (TRN2 allows 1 wait/inst),
    # lowers extended-ISA .instr bytes, register allocation, etc.
    nc.compile()
    return nc


def kernel(class_output, reg_output, class_target, reg_target, class_weights):
    global LAST_RESULTS
    nc = _build_nc()

    def shards(a):
        a = np.ascontiguousarray(np.asarray(a, dtype=np.float32))
        return [
            a[c * NSHARD : (c + 1) * NSHARD].reshape(NT, P, F) for c in range(NCORES)
        ]

    ps = shards(class_output)
    ts = shards(class_target)
    ros = shards(reg_output)
    rts = shards(reg_target)
    in_maps = [
        {"p": ps[c], "t": ts[c], "ro": ros[c], "rt": rts[c]} for c in range(NCORES)
    ]

    res = run_bass_kernel_spmd(nc, in_maps, core_ids=list(range(NCORES)))
    LAST_RESULTS = res

    parts = np.stack([np.asarray(res.results[c]["out"][0]) for c in range(NCORES)])
    tot = parts.sum(axis=0, dtype=np.float64)
    s_tlq, s_lq, s_sq, s_tsq, s_t = tot

    w0 = float(np.asarray(class_weights)[0, 0])
    w1 = float(np.asarray(class_weights)[0, 1])
    # s_lq/s_tlq are sums of ln(q^2) = 2*ln(q)
    class_loss = -(w1 * s_tlq + w0 * (s_lq - s_tlq)) / (2.0 * N)
    cnt = N - s_t
    reg_loss = ((s_sq - s_tsq) / cnt) if cnt > 0 else 0.0
    return np.float32(0.5 * class_loss + 0.5 * reg_loss)


# revision 13
# speedup vs baseline: 1.0073x; 1.0073x over previous
"""Weighted-BCE + masked-MSE loss on 8 Trainium2 cores (pure data parallel).

Math (t in {0,1} exactly): let q = |1 - p - t|  (= p when t=1, 1-p when t=0).
  ln(q^2) = 2*ln(q), so one Square+Ln chain replaces both BCE logs:
    class_sum = -[w1*S_tlq + w0*(S_lq - S_tlq)] / 2,
      S_lq  = sum ln(q^2)   (free via ACT accum),
      S_tlq = sum t*ln(q^2) (one DVE dot)
  masked sq: sum (1-t)*dd^2 = S_sq - S_tsq,  dd = ro - rt
  cnt_zeros = N - S_t  (PE matmul ones.T @ t)
Each core reduces its shard to 5 scalars; host combines and applies weights.

Each core's shard is viewed as [128, 16384] and walked in column chunks:
small chunks at the head (faster pipeline fill) and tail (shorter drain),
2048-wide chunks in the middle.

Engine mix per chunk (DMA is the bottleneck; everything else has slack):
  DMA : p,rt on SP HWDGE; ro on ACT HWDGE; t on Pool SWDGE with f32->bf16 cast
  Pool: s = p + t (its one op)
  ACT : u2 = Square(1-s); lq2 = Ln(u2) [+accum S_lq]; sq = Square(dd) [+accum S_sq]
  DVE : dd = ro - rt; two bf16 product+accum dots (t.lq2, t.sq)
  PE  : count = ones.T @ t accumulated in PSUM; final partition reduce
"""

import os
import sys

for _p in ("/opt/trn_rl_repo", "/root/.axon_site/_ro/trn_rl_repo"):
    if os.path.isdir(_p) and _p not in sys.path:
        sys.path.insert(0, _p)

import numpy as np

import concourse.bacc as bacc
import concourse.mybir as mybir
from concourse import tile
from concourse.bass_utils import run_bass_kernel_spmd

N = 16777216
NCORES = 8
NSHARD = N // NCORES  # 2097152
P = 128
NCOLS = NSHARD // P  # 16384
FMAX = 2048

# Column-chunk schedule: ramp in, cruise, ramp out. Sums to NCOLS.
CHUNKS = [512, 512, 1024] + [2048] * 6 + [1024, 512, 512]
assert sum(CHUNKS) == NCOLS
NC_CHUNKS = len(CHUNKS)

_F32 = mybir.dt.float32
_BF16 = mybir.dt.bfloat16

LAST_RESULTS = None  # test harness peeks at exec_time_ns / trace path


def _build_nc():
    AF = mybir.ActivationFunctionType
    OP = mybir.AluOpType
    AX = mybir.AxisListType

    nc = bacc.Bacc(
        "TRN2", target_bir_lowering=False, debug=False, num_devices=NCORES
    )
    p_d = nc.dram_tensor("p", [P, NCOLS], _F32, kind="ExternalInput")
    t_d = nc.dram_tensor("t", [P, NCOLS], _F32, kind="ExternalInput")
    ro_d = nc.dram_tensor("ro", [P, NCOLS], _F32, kind="ExternalInput")
    rt_d = nc.dram_tensor("rt", [P, NCOLS], _F32, kind="ExternalInput")
    out_d = nc.dram_tensor("out", [1, 5], _F32, kind="ExternalOutput")

    with tile.TileContext(nc) as tc:
        with (
            tc.tile_pool(name="io", bufs=4) as io,
            tc.tile_pool(name="work", bufs=3) as work,
            tc.tile_pool(name="stats", bufs=1) as stats,
            tc.tile_pool(name="psum", bufs=1, space="PSUM") as psum,
        ):
            acc_tlq = stats.tile([P, NC_CHUNKS], _F32)  # sum t*ln(q^2)
            acc_lq = stats.tile([P, NC_CHUNKS], _F32)  # sum ln(q^2)
            acc_sq = stats.tile([P, NC_CHUNKS], _F32)  # sum (ro-rt)^2
            acc_tsq = stats.tile([P, NC_CHUNKS], _F32)  # sum t*(ro-rt)^2

            ones_f = stats.tile([P, 1], _F32)
            nc.vector.memset(ones_f[:], 1.0)
            ones_bf = stats.tile([P, 1], _BF16)
            nc.vector.memset(ones_bf[:], 1.0)
            junk = stats.tile([P, FMAX], _BF16)  # dead dot output, reused

            psum_cnt = psum.tile([1, 512], _F32)

            f0 = 0
            for i, fw in enumerate(CHUNKS):
                sl = slice(f0, f0 + fw)
                tp = io.tile([P, FMAX], _F32, tag="p")
                tt = io.tile([P, FMAX], _BF16, tag="t")
                tro = io.tile([P, FMAX], _F32, tag="ro")
                trt = io.tile([P, FMAX], _F32, tag="rt")
                nc.sync.dma_start(tp[:, 0:fw], p_d[:, sl])
                nc.gpsimd.dma_start(tt[:, 0:fw], t_d[:, sl])  # SWDGE cast
                nc.scalar.dma_start(tro[:, 0:fw], ro_d[:, sl])
                nc.sync.dma_start(trt[:, 0:fw], rt_d[:, sl])

                # Pool: s = p + t
                s = work.tile([P, FMAX], _F32, tag="s")
                nc.gpsimd.tensor_add(s[:, 0:fw], tp[:, 0:fw], tt[:, 0:fw])

                # ACT: u2 = (1-s)^2 = q^2 ; lq2 = ln(q^2) with free accum
                u2 = work.tile([P, FMAX], _BF16, tag="u2")
                nc.scalar.activation(
                    u2[:, 0:fw], s[:, 0:fw], AF.Square, bias=1.0, scale=-1.0
                )
                lq2 = work.tile([P, FMAX], _BF16, tag="lq2")
                nc.scalar.activation(
                    lq2[:, 0:fw], u2[:, 0:fw], AF.Ln,
                    accum_out=acc_lq[:, i : i + 1],
                )

                # DVE: dd = ro - rt; dot t.lq2
                dd = work.tile([P, FMAX], _BF16, tag="dd")
                nc.vector.tensor_sub(dd[:, 0:fw], tro[:, 0:fw], trt[:, 0:fw])
                nc.vector.scalar_tensor_tensor(
                    junk[:, 0:fw], tt[:, 0:fw], 1.0, lq2[:, 0:fw],
                    OP.mult, OP.mult, accum_out=acc_tlq[:, i : i + 1],
                )

                # ACT: sq = dd^2 with free accum; DVE: dot t.sq
                sq = work.tile([P, FMAX], _BF16, tag="sq")
                nc.scalar.activation(
                    sq[:, 0:fw], dd[:, 0:fw], AF.Square,
                    accum_out=acc_sq[:, i : i + 1],
                )
                nc.vector.scalar_tensor_tensor(
                    junk[:, 0:fw], tt[:, 0:fw], 1.0, sq[:, 0:fw],
                    OP.mult, OP.mult, accum_out=acc_tsq[:, i : i + 1],
                )

                # PE: accumulate column-sums of t into psum_cnt
                for c in range(fw // 512):
                    nc.tensor.matmul(
                        psum_cnt[0:1, :],
                        ones_bf[:, 0:1],
                        tt[:, c * 512 : (c + 1) * 512],
                        start=(i == 0 and c == 0),
                        stop=(i == NC_CHUNKS - 1 and c == fw // 512 - 1),
                    )
                f0 += fw

            # Fold per-chunk partials into out[1,5]
            red = stats.tile([P, 4], _F32)
            for j, acc in enumerate((acc_tlq, acc_lq, acc_sq, acc_tsq)):
                nc.vector.tensor_reduce(red[:, j : j + 1], acc[:], AX.X, OP.add)
            psum_fin = psum.tile([1, 4], _F32)
            nc.tensor.matmul(
                psum_fin[0:1, :], ones_f[:, 0:1], red[:, 0:4],
                start=True, stop=True,
            )
            out_sb = stats.tile([P, 8], _F32)
            nc.vector.tensor_scalar_add(out_sb[0:1, 0:4], psum_fin[0:1, :], 0.0)
            nc.vector.tensor_reduce(out_sb[0:1, 4:5], psum_cnt[0:1, :], AX.X, OP.add)
            nc.sync.dma_start(out_d[:], out_sb[0:1, 0:5])

    # Bacc pipeline: splits multi-wait sync (TRN2 allows 1 wait/inst),
    # lowers extended-ISA .instr bytes, register allocation, etc.
    nc.compile()
    return nc


def kernel(class_output, reg_output, class_target, reg_target, class_weights):
    global LAST_RESULTS
    nc = _build_nc()

    def shards(a):
        a = np.ascontiguousarray(np.asarray(a, dtype=np.float32))
        return [
            a[c * NSHARD : (c + 1) * NSHARD].reshape(P, NCOLS) for c in range(NCORES)
        ]

    ps = shards(class_output)
    ts = shards(class_target)
    ros = shards(reg_output)
    rts = shards(reg_target)
    in_maps = [
        {"p": ps[c], "t": ts[c], "ro": ros[c], "rt": rts[c]} for c in range(NCORES)
    ]

    res = run_bass_kernel_spmd(nc, in_maps, core_ids=list(range(NCORES)))
    LAST_RESULTS = res

    parts = np.stack([np.asarray(res.results[c]["out"][0]) for c in range(NCORES)])
    tot = parts.sum(axis=0, dtype=np.float64)
    s_tlq, s_lq, s_sq, s_tsq, s_t = tot

    w0 = float(np.asarray(class_weights)[0, 0])
    w1 = float(np.asarray(class_weights)[0, 1])
    # s_lq/s_tlq are sums of ln(q^2) = 2*ln(q)
    class_loss = -(w1 * s_tlq + w0 * (s_lq - s_tlq)) / (2.0 * N)
    cnt = N - s_t
    reg_loss = ((s_sq - s_tsq) / cnt) if cnt > 0 else 0.0
    return np.float32(0.5 * class_loss + 0.5 * reg_loss)


# revision 14
# speedup vs baseline: 1.2613x; 1.2522x over previous
"""Weighted-BCE + masked-MSE loss on 8 Trainium2 cores (pure data parallel).

Math (t in {0,1} exactly):
  class_sum = sum(bce * w)
            = -(w1 * sum(t*ln p) + w0 * (sum(ln(1-p)) - sum(t*ln(1-p))))
  masked sq = (1-t)*(ro-rt)^2  summed as  sum(dd^2) - sum(t*dd^2)
  cnt_zeros = N - sum(t)
Each core reduces its shard to 6 scalars; host combines and applies weights.

Engine mix per tile (DMA is the bottleneck):
  ACT : tb=Copy(t)->bf16; l1=Ln(p); l0=Ln(1-p)[+accum Sl0]; sq=Square(dd)[+accum]
        (all elementwise outputs in bf16 so the DVE dots run in 2x mode)
  DVE : three bf16 product+accum STT dots: t*l1, t*l0, t*sq
  Pool: dd = ro - rt (the only other 2-input op)
  PE  : count = ones.T @ tb accumulated in PSUM; final partition reduce
"""

import os
import sys

for _p in ("/opt/trn_rl_repo", "/root/.axon_site/_ro/trn_rl_repo"):
    if os.path.isdir(_p) and _p not in sys.path:
        sys.path.insert(0, _p)

import numpy as np

import concourse.bacc as bacc
import concourse.mybir as mybir
from concourse import tile
from concourse.bass_utils import run_bass_kernel_spmd

N = 16777216
NCORES = 8
NSHARD = N // NCORES  # 2097152
P = 128
F = 1024
NT = NSHARD // (P * F)  # 16

_F32 = mybir.dt.float32
_BF16 = mybir.dt.bfloat16

LAST_RESULTS = None  # test harness peeks at exec_time_ns / trace path


def _build_nc():
    AF = mybir.ActivationFunctionType
    OP = mybir.AluOpType
    AX = mybir.AxisListType

    nc = bacc.Bacc(
        "TRN2", target_bir_lowering=False, debug=False, num_devices=NCORES
    )
    p_d = nc.dram_tensor("p", [NT, P, F], _F32, kind="ExternalInput")
    t_d = nc.dram_tensor("t", [NT, P, F], _F32, kind="ExternalInput")
    ro_d = nc.dram_tensor("ro", [NT, P, F], _F32, kind="ExternalInput")
    rt_d = nc.dram_tensor("rt", [NT, P, F], _F32, kind="ExternalInput")
    out_d = nc.dram_tensor("out", [1, 6], _F32, kind="ExternalOutput")

    with tile.TileContext(nc) as tc:
        with (
            tc.tile_pool(name="io", bufs=4) as io,
            tc.tile_pool(name="work", bufs=2) as work,
            tc.tile_pool(name="junkp", bufs=1) as junkp,
            tc.tile_pool(name="stats", bufs=1) as stats,
            tc.tile_pool(name="psum", bufs=1, space="PSUM") as psum,
        ):
            acc_tl1 = stats.tile([P, NT], _F32)  # sum t*ln(p) per tile col
            acc_tl0 = stats.tile([P, NT], _F32)  # sum t*ln(1-p)
            acc_l0 = stats.tile([P, NT], _F32)  # sum ln(1-p)
            acc_sq = stats.tile([P, NT], _F32)  # sum (ro-rt)^2
            acc_tsq = stats.tile([P, NT], _F32)  # sum t*(ro-rt)^2

            ones_f = stats.tile([P, 1], _F32)
            nc.vector.memset(ones_f[:], 1.0)
            ones_bf = stats.tile([P, 1], _BF16)
            nc.vector.memset(ones_bf[:], 1.0)

            psum_cnt = psum.tile([1, 512], _F32)
            NCHUNK = F // 512

            for i in range(NT):
                tp = io.tile([P, F], _F32, tag="p")
                tt = io.tile([P, F], _F32, tag="t")
                tro = io.tile([P, F], _F32, tag="ro")
                trt = io.tile([P, F], _F32, tag="rt")
                nc.sync.dma_start(tp[:], p_d[i, :, :])
                nc.sync.dma_start(tt[:], t_d[i, :, :])
                nc.sync.dma_start(tro[:], ro_d[i, :, :])
                nc.sync.dma_start(trt[:], rt_d[i, :, :])

                # Pool: dd = ro - rt (its one 2-input op)
                dd = work.tile([P, F], _F32, tag="dd")
                nc.gpsimd.tensor_sub(dd[:], tro[:], trt[:])

                # ACT: bf16 cast of t, logs + square; accum_out reduces free
                tb = work.tile([P, F], _BF16, tag="tb")
                nc.scalar.activation(tb[:], tt[:], AF.Copy)
                l1 = work.tile([P, F], _BF16, tag="l1")
                nc.scalar.activation(l1[:], tp[:], AF.Ln)
                l0 = work.tile([P, F], _BF16, tag="l0")
                nc.scalar.activation(
                    l0[:], tp[:], AF.Ln, bias=1.0, scale=-1.0,
                    accum_out=acc_l0[:, i : i + 1],
                )
                sq = work.tile([P, F], _BF16, tag="sq")
                nc.scalar.activation(
                    sq[:], dd[:], AF.Square, accum_out=acc_sq[:, i : i + 1]
                )

                # DVE: fused product+accumulate dots (junk broadcast out)
                junk = junkp.tile([P, 1], _BF16, tag="junk")
                nc.vector.scalar_tensor_tensor(
                    junk[:].broadcast_to([P, F]), tb[:], 1.0, l1[:],
                    OP.mult, OP.mult, accum_out=acc_tl1[:, i : i + 1],
                )
                junk2 = junkp.tile([P, 1], _BF16, tag="junk2")
                nc.vector.scalar_tensor_tensor(
                    junk2[:].broadcast_to([P, F]), tb[:], 1.0, l0[:],
                    OP.mult, OP.mult, accum_out=acc_tl0[:, i : i + 1],
                )
                junk3 = junkp.tile([P, 1], _BF16, tag="junk3")
                nc.vector.scalar_tensor_tensor(
                    junk3[:].broadcast_to([P, F]), tb[:], 1.0, sq[:],
                    OP.mult, OP.mult, accum_out=acc_tsq[:, i : i + 1],
                )

                # PE: accumulate column-sums of t into psum_cnt
                for c in range(NCHUNK):
                    nc.tensor.matmul(
                        psum_cnt[0:1, :],
                        ones_bf[:, 0:1],
                        tb[:, c * 512 : (c + 1) * 512],
                        start=(i == 0 and c == 0),
                        stop=(i == NT - 1 and c == NCHUNK - 1),
                    )

            # Fold per-tile partials into out[1,6]
            red = stats.tile([P, 8], _F32)
            for j, acc in enumerate((acc_tl1, acc_tl0, acc_l0, acc_sq, acc_tsq)):
                nc.vector.tensor_reduce(red[:, j : j + 1], acc[:], AX.X, OP.add)
            psum_fin = psum.tile([1, 5], _F32)
            nc.tensor.matmul(
                psum_fin[0:1, :], ones_f[:, 0:1], red[:, 0:5],
                start=True, stop=True,
            )
            out_sb = stats.tile([P, 8], _F32)
            nc.vector.tensor_scalar_add(out_sb[0:1, 0:5], psum_fin[0:1, :], 0.0)
            nc.vector.tensor_reduce(out_sb[0:1, 5:6], psum_cnt[0:1, :], AX.X, OP.add)
            nc.sync.dma_start(out_d[:], out_sb[0:1, 0:6])

    # Bacc pipeline: splits multi-wait sync (TRN2 allows 1 wait/inst),
    # lowers extended-ISA .instr bytes, register allocation, etc.
    nc.compile()
    return nc


def kernel(class_output, reg_output, class_target, reg_target, class_weights):
    global LAST_RESULTS
    nc = _build_nc()

    def shards(a):
        a = np.ascontiguousarray(np.asarray(a, dtype=np.float32))
        return [
            a[c * NSHARD : (c + 1) * NSHARD].reshape(NT, P, F) for c in range(NCORES)
        ]

    ps = shards(class_output)
    ts = shards(class_target)
    ros = shards(reg_output)
    rts = shards(reg_target)
    in_maps = [
        {"p": ps[c], "t": ts[c], "ro": ros[c], "rt": rts[c]} for c in range(NCORES)
    ]

    res = run_bass_kernel_spmd(nc, in_maps, core_ids=list(range(NCORES)))
    LAST_RESULTS = res

    parts = np.stack([np.asarray(res.results[c]["out"][0]) for c in range(NCORES)])
    tot = parts.sum(axis=0, dtype=np.float64)
    s_tl1, s_tl0, s_l0, s_sq, s_tsq, s_t = tot

    w0 = float(np.asarray(class_weights)[0, 0])
    w1 = float(np.asarray(class_weights)[0, 1])
    class_loss = -(w1 * s_tl1 + w0 * (s_l0 - s_tl0)) / N
    cnt = N - s_t
    reg_loss = ((s_sq - s_tsq) / cnt) if cnt > 0 else 0.0
    return np.float32(0.5 * class_loss + 0.5 * reg_loss)
